# revision 50
# baseline (speedup 1.0000x reference)
"""Trainium2 Bass kernel for nn_EnsemblePolicyHeads (MoE routing head).

Self-contained: accepts FULL inputs, shards batch across the 8 NeuronCores
(data parallel, weights replicated), returns the FULL [8192, 64] output.

Structure (per core, Bc=1024 rows):
  - All bulk DMA rides the single SWDGE ring in exact consumption order
    (the DMA engines are a serial resource; ring order = delivery order):
    z blocks 0-3 (fp32->fp16 cast-DMAs), W1 e0..e1, small weights, W1
    e2..e15, z blocks 4-7. No cross-queue gates needed.
  - nt0 z blocks PE-transposed (fp16) as they land, filling the DMA window;
    nt1 blocks XBAR-transposed straight from the fp16 ztmp into zT.
  - Router logits per 128-block (ba baked in via a K=1 ones-matmul), exp on
    ACT; denominators from fp32 logit transposes, reduced on DVE.
  - exp broadcast: one rep4 matmul per 512-tile builds exp4 (exp in all four
    32-partition quadrants); one DVE stream_shuffle per expert replaces the
    per-expert PE broadcast matmul.
  - Main loop nt-outer: 16 W1 matmuls per (expert, nt), relu on ACT, gating
    multiply on DVE, W2 matmuls deferred by 2 experts, finalize pipelined
    into the next tile's first experts.
"""
import sys

for _p in ("/opt/trn_rl_repo",):
    if _p not in sys.path:
        sys.path.insert(0, _p)


import numpy as np
from contextlib import ExitStack

import concourse.bass as bass
import concourse.tile as tile
from concourse import bacc, mybir
from concourse.masks import make_identity
from concourse.tile_rust import add_dep_helper

F32 = mybir.dt.float32
F16 = mybir.dt.float16
AF = mybir.ActivationFunctionType
ALU = mybir.AluOpType

D = 2048      # input dim
H = 128       # hidden
O = 64        # output dim
E = 16        # num experts
P = 128
KO = D // P   # 16 k-slices
NT_SIZE = 512

W_PAT = "(ko ki) h -> ki ko h"    # i = ko*P + ki
WA_PAT = "(ko ki) e -> ki ko e"

LABELS = {}


def lab(inst, label):
    try:
        LABELS[inst.ins.name] = label
    except Exception:
        pass
    return inst


def build_kernel(Bc: int):
    assert Bc % NT_SIZE == 0
    NT = Bc // NT_SIZE        # 2
    SUBS = NT_SIZE // P       # 4
    NBLK = Bc // P            # 8

    nc = bacc.Bacc("TRN2", target_bir_lowering=False, debug=False)
    z_ap = nc.dram_tensor("z", [Bc, D], F32, kind="ExternalInput").ap()
    W1_ap = nc.dram_tensor("W1", [E, D, H], F32, kind="ExternalInput").ap()
    b1_ap = nc.dram_tensor("b1", [E, H], F32, kind="ExternalInput").ap()
    W2_ap = nc.dram_tensor("W2", [E, H, O], F32, kind="ExternalInput").ap()
    b2_ap = nc.dram_tensor("b2", [E, O], F32, kind="ExternalInput").ap()
    Wa_ap = nc.dram_tensor("Wa", [D, E], F32, kind="ExternalInput").ap()
    ba_ap = nc.dram_tensor("ba", [E], F32, kind="ExternalInput").ap()
    out_ap = nc.dram_tensor("out", [Bc, O], F32, kind="ExternalOutput").ap()

    with tile.TileContext(nc) as tc, ExitStack() as ctx:
        persist = ctx.enter_context(tc.tile_pool(name="persist", bufs=1))
        ztmp_pool = ctx.enter_context(tc.tile_pool(name="ztmp", bufs=4))
        lgt_pool = ctx.enter_context(tc.tile_pool(name="lgt", bufs=2))
        t_pool = ctx.enter_context(tc.tile_pool(name="t", bufs=3))
        shuf_pool = ctx.enter_context(tc.tile_pool(name="shuf", bufs=3))
        hm_pool = ctx.enter_context(tc.tile_pool(name="hm", bufs=4))
        res_pool = ctx.enter_context(tc.tile_pool(name="res", bufs=2))
        outsb_pool = ctx.enter_context(tc.tile_pool(name="outsb", bufs=4))
        psA = ctx.enter_context(tc.tile_pool(name="psA", bufs=2, space="PSUM"))
        psB = ctx.enter_context(tc.tile_pool(name="psB", bufs=1, space="PSUM"))
        psC = ctx.enter_context(tc.tile_pool(name="psC", bufs=2, space="PSUM"))
        psD = ctx.enter_context(tc.tile_pool(name="psD", bufs=2, space="PSUM"))
        psE = ctx.enter_context(tc.tile_pool(name="psE", bufs=1, space="PSUM"))

        # ---- persistent tiles ----
        zT = persist.tile([P, KO, Bc], F16)
        W1bf = persist.tile([P, KO, E, H], F16)
        Wabf = persist.tile([P, KO, E], F16)
        Wasb = persist.tile([P, KO, E], F32)
        W2bf = persist.tile([P, E, O], F16)
        b2bf = persist.tile([E, O], F16)
        ba16 = persist.tile([1, E], F16)
        ones = persist.tile([1, NT_SIZE], F16)
        b1sb = persist.tile([E, H], F32)
        b1T = persist.tile([P, E], F32)
        expT0 = persist.tile([E, Bc], F16)
        exp4 = persist.tile([P, Bc], F16)
        attn_be = persist.tile([P, NBLK, E], F32)
        denomT = persist.tile([P, NBLK], F32)
        recipT = persist.tile([P, NBLK], F32)
        id_f32 = persist.tile([P, P], F32)
        id_bf = persist.tile([P, P], F16)
        rep4 = persist.tile([E, 4, 32], F16)  # rep4[e,q,w] = (w == e)

        ztmps = {}

        def load_z_block(blk, parts=1):
            zt = ztmp_pool.tile([P, D], F16, tag="ztmp")
            rows = slice(blk * P, (blk + 1) * P)
            w = D // parts
            first = zd_last = None
            for i in range(parts):
                zd_last = lab(
                    nc.gpsimd.dma_start(zt[:, i * w:(i + 1) * w],
                                        z_ap[rows, i * w:(i + 1) * w]),
                    f"zdma{blk}.{i}" if parts > 1 else f"zdma{blk}")
                if first is None:
                    first = zd_last
            ztmps[blk] = zt
            return first, zd_last

        def pe_transpose_block(blk):
            zt = ztmps.pop(blk)
            for pr in range(KO // 2):
                ps = psD.tile([P, 2 * P], F16, tag="ps_tr")
                for h2 in range(2):
                    ko = 2 * pr + h2
                    lab(nc.tensor.transpose(
                        ps[:, h2 * P:(h2 + 1) * P],
                        zt[:, ko * P:(ko + 1) * P], id_bf[:]), f"ztr{blk}.{pr}.{h2}")
                dst = zT[:, 2 * pr:2 * pr + 2, blk * P:(blk + 1) * P]
                src = ps[:].rearrange("p (k b) -> p k b", k=2)
                if pr % 2 == 0:
                    nc.scalar.copy(dst, src)
                else:
                    nc.vector.tensor_copy(dst, src)

        def xbar_transpose_block(blk):
            # SP queue: idle until the finalize out-DMAs, so each trigger
            # fires the moment its z block lands.
            zt = ztmps.pop(blk)
            lab(nc.sync.dma_start_transpose(
                zT[:, :, blk * P:(blk + 1) * P], zt[:]), f"xbar{blk}")

        def logits_block(blk):
            bs = slice(blk * P, (blk + 1) * P)
            ps_lf = psB.tile([P, NT_SIZE], F32, tag="ps_l")
            ps_l = ps_lf[:E, :P]
            lab(nc.tensor.matmul(ps_l, ba16[:], ones[:, :P],
                                 start=True, stop=False), f"balgt{blk}")
            for ko in range(KO):
                lab(nc.tensor.matmul(
                    ps_l, Wabf[:, ko, :], zT[:, ko, bs],
                    start=False, stop=(ko == KO - 1)), f"lgt{blk}.{ko}")
            nc.scalar.activation(expT0[:, bs], ps_l, AF.Exp)
            lgt_sb = lgt_pool.tile([E, P], F32, tag="lgt")
            nc.scalar.copy(lgt_sb[:], ps_l)
            ps_t = psE.tile([P, NT_SIZE], F32, tag="ps_e4")
            lab(nc.tensor.transpose(ps_t[:, :E], lgt_sb[:], id_f32[:E, :E]),
                f"dtr{blk}")
            nc.scalar.activation(attn_be[:, blk, :], ps_t[:, :E], AF.Exp)

        def denom_nt(nt):
            nts = slice(nt * SUBS, (nt + 1) * SUBS)
            nc.vector.reduce_sum(
                denomT[:, nts, None], attn_be[:, nts, :], axis=mybir.AxisListType.X)
            nc.vector.reciprocal(recipT[:, nts], denomT[:, nts])

        def exp4_nt(nt):
            bs = slice(nt * NT_SIZE, (nt + 1) * NT_SIZE)
            ps_e4 = psE.tile([P, NT_SIZE], F32, tag="ps_e4")
            lab(nc.tensor.matmul(ps_e4[:], rep4[:], expT0[:, bs],
                                 start=True, stop=True), f"rep4.{nt}")
            nc.scalar.copy(exp4[:, bs], ps_e4[:])

        # ---- tiny loads off the ring: b1 on SP, Wa on scalar HWDGE (gated
        # behind the first z block so z keeps the engines at t0).
        nc.sync.dma_start(b1sb[:], b1_ap[:])
        nc.vector.memset(ones, 1.0)

        # SWDGE ring in consumption order, interleaved with Pool setup work.
        zd = {}
        z0_first, zd[0] = load_z_block(0, parts=2)
        lab(nc.gpsimd.dma_start(ba16[:], ba_ap[None, :]), "badma")
        make_identity(nc, id_bf)
        _, zd[1] = load_z_block(1, parts=2)
        make_identity(nc, id_f32)
        _, zd[2] = load_z_block(2)
        nc.gpsimd.memset(rep4, 0.0)
        nc.gpsimd.affine_select(
            out=rep4, in_=rep4, compare_op=ALU.not_equal, fill=1.0,
            base=0, pattern=[[0, 4], [-1, 32]], channel_multiplier=1)

        wa_d = lab(nc.scalar.dma_start(Wasb[:], Wa_ap.rearrange(WA_PAT, ki=P)),
                   "wadma")
        add_dep_helper(wa_d.ins, z0_first.ins, reason="z block 0 first")
        nc.vector.tensor_copy(Wabf[:], Wasb[:])

        def w1_dma(e):
            return lab(nc.gpsimd.dma_start(
                W1bf[:, :, e, :], W1_ap[e].rearrange(W_PAT, ki=P)), f"w1dma{e}")

        w1d = {}
        w1d[0] = w1_dma(0)
        _, zd[3] = load_z_block(3)
        w1d[1] = w1_dma(1)
        lab(nc.gpsimd.dma_start(b2bf[:], b2_ap[:]), "b2dma")
        lab(nc.gpsimd.dma_start(
            W2bf[:], W2_ap.rearrange("e h o -> h e o")), "w2dma")
        for e in range(2, E):
            w1d[e] = w1_dma(e)
        for blk in (4, 5, 6, 7):
            _, zd[blk] = load_z_block(blk)
            xbar_transpose_block(blk)

        # ---- startup: transpose + router for nt0 blocks as they land.
        # T3 runs ahead of L2 so its zT copies finish before expert 0 needs
        # them; block 3's router runs after expert 0's W1 group.
        for blk in range(SUBS - 1):
            pe_transpose_block(blk)
            if blk == 0:
                # b1 -> b1T [H, E] via PE transpose (needed from relu e0)
                ps_b1 = psE.tile([P, NT_SIZE], F32, tag="ps_e4")
                nc.tensor.transpose(ps_b1[:, :E], b1sb[:], id_f32[:E, :E])
                nc.scalar.copy(b1T[:], ps_b1[:, :E])
            if blk == SUBS - 2:
                pe_transpose_block(SUBS - 1)
            logits_block(blk)

        # ---- main loop ----
        pend_w2 = []      # deque of (e, hm, ps_o) deferred W2 matmuls
        pend_fin = None   # (nt, ps_o) to finalize during next nt

        def flush_w2(keep):
            while len(pend_w2) > keep:
                pe_, phm, po = pend_w2.pop(0)
                lab(nc.tensor.matmul(po[:], W2bf[:, pe_, :], phm[:],
                                     start=False, stop=(pe_ == E - 1)),
                    f"w2mm{pe_}")

        def finalize_nt(nt, ps_o):
            res = res_pool.tile([O, NT_SIZE], F32)
            outsb = outsb_pool.tile([P, SUBS, O], F32)
            for sub in range(SUBS):
                blk = nt * SUBS + sub
                cs = slice(sub * P, (sub + 1) * P)
                nc.scalar.copy(res[:, cs], ps_o[:, cs])
                pool, tag = ((psE, "ps_e4"), (psB, "ps_l"))[sub % 2]
                ps_t2 = pool.tile([P, NT_SIZE], F32, tag=tag)
                lab(nc.tensor.transpose(
                    ps_t2[:, :O], res[:, cs], id_f32[:O, :O]), f"ftr{blk}")
                nc.vector.tensor_scalar_mul(outsb[:, sub, :], ps_t2[:, :O],
                                            recipT[:, blk:blk + 1])
            nc.sync.dma_start(
                out_ap[nt * NT_SIZE:(nt + 1) * NT_SIZE, :].rearrange(
                    "(s p) o -> p s o", p=P),
                outsb[:])

        for nt in range(NT):
            bs = slice(nt * NT_SIZE, (nt + 1) * NT_SIZE)
            ps_o = psC.tile([O, NT_SIZE], F32, tag="ps_o")
            for e in range(E):
                ps_h = psA.tile([P, NT_SIZE], F32, tag="ps_h")
                for ko in range(KO):
                    lab(nc.tensor.matmul(
                        ps_h[:], W1bf[:, ko, e, :], zT[:, ko, bs],
                        start=(ko == 0), stop=(ko == KO - 1)), f"w1mm{nt}.{e}.{ko}")
                if e == 0:
                    # deferred router work for the tile we just started
                    if nt == 0:
                        logits_block(3)
                        denom_nt(0)
                        exp4_nt(0)
                    else:
                        logits_block(6)
                        logits_block(7)
                        denom_nt(1)
                        exp4_nt(1)
                if e == 1:
                    # b2 opens the ps_o accumulation group (runs before w2mm0)
                    lab(nc.tensor.matmul(ps_o[:], b2bf[:], expT0[:, bs],
                                         start=True, stop=False), f"b2mm{nt}")
                    if pend_fin is not None:
                        finalize_nt(*pend_fin)
                        pend_fin = None
                flush_w2(2 if e < E - 1 else 0)
                if nt == 0:
                    # nt1 routers once their zT slices exist
                    if e == 14:
                        logits_block(4)
                    if e == 15:
                        logits_block(5)
                shuf = shuf_pool.tile([P, NT_SIZE], F16)
                lab(nc.vector.stream_shuffle(shuf[:], exp4[:, bs], mask=[e] * 32),
                    f"shuf{nt}.{e}")
                t = t_pool.tile([P, NT_SIZE], F16)
                nc.scalar.activation(t[:], ps_h[:], AF.Relu, bias=b1T[:, e:e + 1])
                hm = hm_pool.tile([P, NT_SIZE], F16)
                nc.vector.tensor_tensor(hm[:], t[:], shuf[:], ALU.mult)
                pend_w2.append((e, hm, ps_o))
            flush_w2(0)
            pend_fin = (nt, ps_o)
        finalize_nt(*pend_fin)

    nc.compile()
    try:
        import json
        with open("/tmp/kernel_labels.json", "w") as f:
            json.dump(LABELS, f)
    except Exception:
        pass
    return nc


# ---------------------------------------------------------------------------
# Harness entry point
# ---------------------------------------------------------------------------
N_CORES = 8
B_TOTAL = 8192
BC = B_TOTAL // N_CORES

_nc_cache = {}


def _get_nc():
    if "nc" not in _nc_cache:
        _nc_cache["nc"] = build_kernel(BC)
    return _nc_cache["nc"]


def kernel(z_i, W1, b1, W2, b2, Wa, ba):
    from concourse.bass_utils import run_bass_kernel_spmd

    z = np.ascontiguousarray(np.asarray(z_i, dtype=np.float32).reshape(B_TOTAL, D))
    W1 = np.ascontiguousarray(np.asarray(W1, dtype=np.float32))
    b1 = np.ascontiguousarray(np.asarray(b1, dtype=np.float32))
    W2 = np.ascontiguousarray(np.asarray(W2, dtype=np.float32))
    b2 = np.ascontiguousarray(np.asarray(b2, dtype=np.float32))
    Wa = np.ascontiguousarray(np.asarray(Wa, dtype=np.float32))
    ba = np.ascontiguousarray(np.asarray(ba, dtype=np.float32))

    nc = _get_nc()
    in_maps = [
        dict(z=z[c * BC:(c + 1) * BC], W1=W1, b1=b1, W2=W2, b2=b2, Wa=Wa, ba=ba)
        for c in range(N_CORES)
    ]
    res = run_bass_kernel_spmd(nc, in_maps, core_ids=list(range(N_CORES)))
    return np.concatenate([res.results[c]["out"] for c in range(N_CORES)], axis=0)


# revision 62
# speedup vs baseline: 1.0045x; 1.0045x over previous
"""Trainium2 Bass kernel for nn_EnsemblePolicyHeads (MoE routing head).

Self-contained: accepts FULL inputs, shards batch across the 8 NeuronCores
(data parallel, weights replicated), returns the FULL [8192, 64] output.

Structure (per core, Bc=1024 rows):
  - All bulk DMA rides the single SWDGE ring in exact consumption order
    (the DMA engines are a serial resource; ring order = delivery order):
    z blocks 0-3 (fp32->fp16 cast-DMAs), W1 e0..e1, small weights, W1
    e2..e15, z blocks 4-7. No cross-queue gates needed.
  - nt0 z blocks PE-transposed (fp16) as they land, filling the DMA window;
    nt1 blocks XBAR-transposed straight from the fp16 ztmp into zT.
  - Router logits per 128-block; ba applied as the ACT bias (Exp for the
    numerator, Identity-add for the fp32 logit copy whose transpose feeds
    the DVE-reduced denominators).
  - exp broadcast: one rep4 matmul per 512-tile builds exp4 (exp in all four
    32-partition quadrants); one DVE stream_shuffle per expert replaces the
    per-expert PE broadcast matmul.
  - Main loop nt-outer: 16 W1 matmuls per (expert, nt), relu on ACT, gating
    multiply on DVE, W2 matmuls deferred by 2 experts, finalize pipelined
    into the next tile's first experts.
"""
import sys

for _p in ("/opt/trn_rl_repo",):
    if _p not in sys.path:
        sys.path.insert(0, _p)


import numpy as np
from contextlib import ExitStack

import concourse.bass as bass
import concourse.tile as tile
from concourse import bacc, mybir
from concourse.masks import make_identity
from concourse.tile_rust import add_dep_helper

F32 = mybir.dt.float32
F16 = mybir.dt.float16
AF = mybir.ActivationFunctionType
ALU = mybir.AluOpType

D = 2048      # input dim
H = 128       # hidden
O = 64        # output dim
E = 16        # num experts
P = 128
KO = D // P   # 16 k-slices
NT_SIZE = 512

W_PAT = "(ko ki) h -> ki ko h"    # i = ko*P + ki
WA_PAT = "(ko ki) e -> ki ko e"

LABELS = {}


def lab(inst, label):
    try:
        LABELS[inst.ins.name] = label
    except Exception:
        pass
    return inst


def build_kernel(Bc: int):
    assert Bc % NT_SIZE == 0
    NT = Bc // NT_SIZE        # 2
    SUBS = NT_SIZE // P       # 4
    NBLK = Bc // P            # 8

    nc = bacc.Bacc("TRN2", target_bir_lowering=False, debug=False)
    z_ap = nc.dram_tensor("z", [Bc, D], F32, kind="ExternalInput").ap()
    W1_ap = nc.dram_tensor("W1", [E, D, H], F32, kind="ExternalInput").ap()
    b1_ap = nc.dram_tensor("b1", [E, H], F32, kind="ExternalInput").ap()
    W2_ap = nc.dram_tensor("W2", [E, H, O], F32, kind="ExternalInput").ap()
    b2_ap = nc.dram_tensor("b2", [E, O], F32, kind="ExternalInput").ap()
    Wa_ap = nc.dram_tensor("Wa", [D, E], F32, kind="ExternalInput").ap()
    ba_ap = nc.dram_tensor("ba", [E], F32, kind="ExternalInput").ap()
    out_ap = nc.dram_tensor("out", [Bc, O], F32, kind="ExternalOutput").ap()

    with tile.TileContext(nc) as tc, ExitStack() as ctx:
        persist = ctx.enter_context(tc.tile_pool(name="persist", bufs=1))
        ztmp_pool = ctx.enter_context(tc.tile_pool(name="ztmp", bufs=4))
        lgt_pool = ctx.enter_context(tc.tile_pool(name="lgt", bufs=2))
        t_pool = ctx.enter_context(tc.tile_pool(name="t", bufs=3))
        shuf_pool = ctx.enter_context(tc.tile_pool(name="shuf", bufs=3))
        hm_pool = ctx.enter_context(tc.tile_pool(name="hm", bufs=4))
        res_pool = ctx.enter_context(tc.tile_pool(name="res", bufs=2))
        outsb_pool = ctx.enter_context(tc.tile_pool(name="outsb", bufs=4))
        psA = ctx.enter_context(tc.tile_pool(name="psA", bufs=2, space="PSUM"))
        psB = ctx.enter_context(tc.tile_pool(name="psB", bufs=1, space="PSUM"))
        psC = ctx.enter_context(tc.tile_pool(name="psC", bufs=2, space="PSUM"))
        psD = ctx.enter_context(tc.tile_pool(name="psD", bufs=2, space="PSUM"))
        psE = ctx.enter_context(tc.tile_pool(name="psE", bufs=1, space="PSUM"))

        # ---- persistent tiles ----
        zT = persist.tile([P, KO, Bc], F16)
        W1bf = persist.tile([P, KO, E, H], F16)
        Wabf = persist.tile([P, KO, E], F16)
        Wasb = persist.tile([P, KO, E], F32)
        W2bf = persist.tile([P, E, O], F16)
        b2bf = persist.tile([E, O], F16)
        ba_sb = persist.tile([E, 1], F32)
        b1sb = persist.tile([E, H], F32)
        b1T = persist.tile([P, E], F32)
        expT0 = persist.tile([E, Bc], F16)
        exp4 = persist.tile([P, Bc], F16)
        attn_be = persist.tile([P, NBLK, E], F32)
        denomT = persist.tile([P, NBLK], F32)
        recipT = persist.tile([P, NBLK], F32)
        id_f32 = persist.tile([P, P], F32)
        id_bf = persist.tile([P, P], F16)
        rep4 = persist.tile([E, 4, 32], F16)  # rep4[e,q,w] = (w == e)

        ztmps = {}

        def load_z_block(blk, parts=1):
            zt = ztmp_pool.tile([P, D], F16, tag="ztmp")
            rows = slice(blk * P, (blk + 1) * P)
            w = D // parts
            first = zd_last = None
            for i in range(parts):
                zd_last = lab(
                    nc.gpsimd.dma_start(zt[:, i * w:(i + 1) * w],
                                        z_ap[rows, i * w:(i + 1) * w]),
                    f"zdma{blk}.{i}" if parts > 1 else f"zdma{blk}")
                if first is None:
                    first = zd_last
            ztmps[blk] = zt
            return first, zd_last

        def pe_transpose_block(blk):
            zt = ztmps.pop(blk)
            for pr in range(KO // 2):
                ps = psD.tile([P, 2 * P], F16, tag="ps_tr")
                for h2 in range(2):
                    ko = 2 * pr + h2
                    lab(nc.tensor.transpose(
                        ps[:, h2 * P:(h2 + 1) * P],
                        zt[:, ko * P:(ko + 1) * P], id_bf[:]), f"ztr{blk}.{pr}.{h2}")
                dst = zT[:, 2 * pr:2 * pr + 2, blk * P:(blk + 1) * P]
                src = ps[:].rearrange("p (k b) -> p k b", k=2)
                if pr % 2 == 0:
                    nc.scalar.copy(dst, src)
                else:
                    nc.vector.tensor_copy(dst, src)

        def xbar_transpose_block(blk):
            # SP queue: idle until the finalize out-DMAs, so each trigger
            # fires the moment its z block lands.
            zt = ztmps.pop(blk)
            lab(nc.sync.dma_start_transpose(
                zT[:, :, blk * P:(blk + 1) * P], zt[:]), f"xbar{blk}")

        def logits_block(blk):
            bs = slice(blk * P, (blk + 1) * P)
            ps_lf = psB.tile([P, NT_SIZE], F32, tag="ps_l")
            ps_l = ps_lf[:E, :P]
            for ko in range(KO):
                lab(nc.tensor.matmul(
                    ps_l, Wabf[:, ko, :], zT[:, ko, bs],
                    start=(ko == 0), stop=(ko == KO - 1)), f"lgt{blk}.{ko}")
            nc.scalar.activation(expT0[:, bs], ps_l, AF.Exp, bias=ba_sb[:])
            lgt_sb = lgt_pool.tile([E, P], F32, tag="lgt")
            nc.scalar.add(lgt_sb[:], ps_l, ba_sb[:])
            ps_t = psE.tile([P, NT_SIZE], F32, tag="ps_e4")
            lab(nc.tensor.transpose(ps_t[:, :E], lgt_sb[:], id_f32[:E, :E]),
                f"dtr{blk}")
            nc.scalar.activation(attn_be[:, blk, :], ps_t[:, :E], AF.Exp)

        def denom_nt(nt):
            nts = slice(nt * SUBS, (nt + 1) * SUBS)
            nc.vector.reduce_sum(
                denomT[:, nts, None], attn_be[:, nts, :], axis=mybir.AxisListType.X)
            nc.vector.reciprocal(recipT[:, nts], denomT[:, nts])

        def exp4_nt(nt):
            bs = slice(nt * NT_SIZE, (nt + 1) * NT_SIZE)
            ps_e4 = psE.tile([P, NT_SIZE], F32, tag="ps_e4")
            lab(nc.tensor.matmul(ps_e4[:], rep4[:], expT0[:, bs],
                                 start=True, stop=True), f"rep4.{nt}")
            nc.scalar.copy(exp4[:, bs], ps_e4[:])

        # ---- tiny loads off the ring: b1/ba on SP, Wa on scalar HWDGE
        # (gated behind the first z block so z keeps the engines at t0).
        nc.sync.dma_start(b1sb[:], b1_ap[:])
        nc.sync.dma_start(ba_sb[:], ba_ap[:, None])

        # SWDGE ring in consumption order, interleaved with Pool setup work.
        zd = {}
        z0_first, zd[0] = load_z_block(0, parts=2)
        make_identity(nc, id_bf)
        _, zd[1] = load_z_block(1, parts=2)
        make_identity(nc, id_f32)
        _, zd[2] = load_z_block(2)
        nc.gpsimd.memset(rep4, 0.0)
        nc.gpsimd.affine_select(
            out=rep4, in_=rep4, compare_op=ALU.not_equal, fill=1.0,
            base=0, pattern=[[0, 4], [-1, 32]], channel_multiplier=1)

        wa_d = lab(nc.scalar.dma_start(Wasb[:], Wa_ap.rearrange(WA_PAT, ki=P)),
                   "wadma")
        add_dep_helper(wa_d.ins, z0_first.ins, reason="z block 0 first")
        nc.vector.tensor_copy(Wabf[:], Wasb[:])

        def w1_dma(e):
            return lab(nc.gpsimd.dma_start(
                W1bf[:, :, e, :], W1_ap[e].rearrange(W_PAT, ki=P)), f"w1dma{e}")

        w1d = {}
        w1d[0] = w1_dma(0)
        _, zd[3] = load_z_block(3)
        w1d[1] = w1_dma(1)
        lab(nc.gpsimd.dma_start(b2bf[:], b2_ap[:]), "b2dma")
        lab(nc.gpsimd.dma_start(
            W2bf[:], W2_ap.rearrange("e h o -> h e o")), "w2dma")
        for e in range(2, E):
            w1d[e] = w1_dma(e)
        for blk in (4, 5, 6, 7):
            _, zd[blk] = load_z_block(blk)
            xbar_transpose_block(blk)

        # ---- startup: transpose + router for nt0 blocks as they land.
        # T3 runs ahead of L2 so its zT copies finish before expert 0 needs
        # them; block 3's router runs after expert 0's W1 group.
        for blk in range(SUBS - 1):
            pe_transpose_block(blk)
            if blk == 0:
                # b1 -> b1T [H, E] via PE transpose (needed from relu e0)
                ps_b1 = psE.tile([P, NT_SIZE], F32, tag="ps_e4")
                nc.tensor.transpose(ps_b1[:, :E], b1sb[:], id_f32[:E, :E])
                nc.scalar.copy(b1T[:], ps_b1[:, :E])
            if blk == SUBS - 2:
                pe_transpose_block(SUBS - 1)
            logits_block(blk)

        # ---- main loop ----
        pend_w2 = []      # deque of (e, hm, ps_o) deferred W2 matmuls
        pend_fin = None   # (nt, ps_o) to finalize during next nt

        def flush_w2(keep):
            while len(pend_w2) > keep:
                pe_, phm, po = pend_w2.pop(0)
                lab(nc.tensor.matmul(po[:], W2bf[:, pe_, :], phm[:],
                                     start=False, stop=(pe_ == E - 1)),
                    f"w2mm{pe_}")

        def finalize_nt(nt, ps_o):
            res = res_pool.tile([O, NT_SIZE], F32)
            outsb = outsb_pool.tile([P, SUBS, O], F32)
            for sub in range(SUBS):
                blk = nt * SUBS + sub
                cs = slice(sub * P, (sub + 1) * P)
                nc.scalar.copy(res[:, cs], ps_o[:, cs])
                pool, tag = ((psE, "ps_e4"), (psB, "ps_l"))[sub % 2]
                ps_t2 = pool.tile([P, NT_SIZE], F32, tag=tag)
                lab(nc.tensor.transpose(
                    ps_t2[:, :O], res[:, cs], id_f32[:O, :O]), f"ftr{blk}")
                nc.vector.tensor_scalar_mul(outsb[:, sub, :], ps_t2[:, :O],
                                            recipT[:, blk:blk + 1])
            nc.sync.dma_start(
                out_ap[nt * NT_SIZE:(nt + 1) * NT_SIZE, :].rearrange(
                    "(s p) o -> p s o", p=P),
                outsb[:])

        for nt in range(NT):
            bs = slice(nt * NT_SIZE, (nt + 1) * NT_SIZE)
            ps_o = psC.tile([O, NT_SIZE], F32, tag="ps_o")
            for e in range(E):
                ps_h = psA.tile([P, NT_SIZE], F32, tag="ps_h")
                for ko in range(KO):
                    lab(nc.tensor.matmul(
                        ps_h[:], W1bf[:, ko, e, :], zT[:, ko, bs],
                        start=(ko == 0), stop=(ko == KO - 1)), f"w1mm{nt}.{e}.{ko}")
                if e == 0:
                    # deferred router work for the tile we just started
                    if nt == 0:
                        logits_block(3)
                        denom_nt(0)
                        exp4_nt(0)
                    else:
                        logits_block(6)
                        logits_block(7)
                        denom_nt(1)
                        exp4_nt(1)
                if e == 1:
                    # b2 opens the ps_o accumulation group (runs before w2mm0)
                    lab(nc.tensor.matmul(ps_o[:], b2bf[:], expT0[:, bs],
                                         start=True, stop=False), f"b2mm{nt}")
                    if pend_fin is not None:
                        finalize_nt(*pend_fin)
                        pend_fin = None
                flush_w2(2 if e < E - 1 else 0)
                if nt == 0:
                    # nt1 routers once their zT slices exist
                    if e == 14:
                        logits_block(4)
                    if e == 15:
                        logits_block(5)
                shuf = shuf_pool.tile([P, NT_SIZE], F16)
                lab(nc.vector.stream_shuffle(shuf[:], exp4[:, bs], mask=[e] * 32),
                    f"shuf{nt}.{e}")
                t = t_pool.tile([P, NT_SIZE], F16)
                nc.scalar.activation(t[:], ps_h[:], AF.Relu, bias=b1T[:, e:e + 1])
                hm = hm_pool.tile([P, NT_SIZE], F16)
                nc.vector.tensor_tensor(hm[:], t[:], shuf[:], ALU.mult)
                pend_w2.append((e, hm, ps_o))
            flush_w2(0)
            pend_fin = (nt, ps_o)
        finalize_nt(*pend_fin)

    nc.compile()
    try:
        import json
        with open("/tmp/kernel_labels.json", "w") as f:
            json.dump(LABELS, f)
    except Exception:
        pass
    return nc


# ---------------------------------------------------------------------------
# Harness entry point
# ---------------------------------------------------------------------------
N_CORES = 8
B_TOTAL = 8192
BC = B_TOTAL // N_CORES

_nc_cache = {}


def _get_nc():
    if "nc" not in _nc_cache:
        _nc_cache["nc"] = build_kernel(BC)
    return _nc_cache["nc"]


def kernel(z_i, W1, b1, W2, b2, Wa, ba):
    from concourse.bass_utils import run_bass_kernel_spmd

    z = np.ascontiguousarray(np.asarray(z_i, dtype=np.float32).reshape(B_TOTAL, D))
    W1 = np.ascontiguousarray(np.asarray(W1, dtype=np.float32))
    b1 = np.ascontiguousarray(np.asarray(b1, dtype=np.float32))
    W2 = np.ascontiguousarray(np.asarray(W2, dtype=np.float32))
    b2 = np.ascontiguousarray(np.asarray(b2, dtype=np.float32))
    Wa = np.ascontiguousarray(np.asarray(Wa, dtype=np.float32))
    ba = np.ascontiguousarray(np.asarray(ba, dtype=np.float32))

    nc = _get_nc()
    in_maps = [
        dict(z=z[c * BC:(c + 1) * BC], W1=W1, b1=b1, W2=W2, b2=b2, Wa=Wa, ba=ba)
        for c in range(N_CORES)
    ]
    res = run_bass_kernel_spmd(nc, in_maps, core_ids=list(range(N_CORES)))
    return np.concatenate([res.results[c]["out"] for c in range(N_CORES)], axis=0)
